# revision 6
# baseline (speedup 1.0000x reference)
"""DiagonalLinear on 8 TRN2 NeuronCores.

y = x * clip(diagonal, -0.95, 0.95)  with x [16384, 8192] f32, diagonal [8192] f32.

Purely memory-bound (elementwise): per-core HBM traffic is the whole cost, and
the per-NeuronCore HBM limit is ~358 GB/s shared by loads+stores. The f32
version moves 132 MiB/core (~390 us floor). The rel-err budget (2e-2) is ~10x
wider than a bf16 round-trip (~2e-3), so x is quantized to bf16 on the host,
the device streams bf16 in and out (64 MiB/core), and the host upcasts the
result to f32. That halves HBM bytes -> ~190 us floor.

Data-parallel: x sharded along batch (2048 rows/core). The diagonal is NOT
replicated through HBM (a [128, 8192] bf16 load would cost 2 MiB ~ 6 us of
HBM): instead a single [1, 8192] row (16 KiB) is loaded and broadcast across
the 128 partitions by the otherwise-idle PE (ones[1,128].T @ d[1,512-chunks]
into PSUM), with the clamp fused into the PSUM -> SBUF copy on DVE.

Steady state: 16 tiles of [128, 8192] bf16 (2 MiB contiguous DMAs) stream
through a load -> DVE mul -> store pipeline. Loads issue on the SP HWDGE ring,
stores on the ACT HWDGE ring; both rings feed the same 16 SDMA engines which
round-robin at packet granularity, so the streams share HBM bandwidth without
serializing. Trace shows the DMA union saturated at ~358 GB/s throughout.

Raw Bass (no TileContext): this walrus build rejects Tile's multi-wait
kernel-tail drain, and manual sync keeps every instruction at <=1 sem wait.
The kernel ends with barrier -> sem reset -> barrier so the NEFF is safely
re-executable (NTFF profiling reruns it with leftover sem values otherwise);
the reset is batched into one dma_reset + one sem_clear over the whole sem
range to keep the tail short.
"""

import numpy as np
import ml_dtypes

import concourse.bass as bass
import concourse.mybir as mybir
from concourse.bass_utils import run_bass_kernel_spmd

BATCH = 16384
LATENT = 8192
N_CORES = 8
ROWS_PER_CORE = BATCH // N_CORES  # 2048
P = 128
N_TILES = ROWS_PER_CORE // P  # 16
NBUF = 8
MMCOL = 512  # PE moving-data free-dim max
PSCOL = 4096  # PSUM tensor width (8 banks), half of LATENT

BF16 = mybir.dt.bfloat16
NP_BF16 = ml_dtypes.bfloat16

_NC_CACHE: dict[str, bass.Bass] = {}


def _build() -> bass.Bass:
    if "nc" in _NC_CACHE:
        return _NC_CACHE["nc"]

    nc = bass.Bass()
    x = nc.dram_tensor("x", [ROWS_PER_CORE, LATENT], BF16, kind="ExternalInput")
    d = nc.dram_tensor("diagonal", [1, LATENT], BF16, kind="ExternalInput")
    out = nc.dram_tensor("out", [ROWS_PER_CORE, LATENT], BF16, kind="ExternalOutput")

    xt = x.rearrange("(n p) m -> n p m", p=P)  # [16, 128, 8192]
    ot = out.rearrange("(n p) m -> n p m", p=P)

    def buf(i):
        b = i % NBUF
        return slice(b * LATENT, (b + 1) * LATENT)

    with (
        nc.sbuf_tensor([P, NBUF * LATENT], BF16) as xbuf,
        nc.sbuf_tensor([P, LATENT], BF16) as dbc,  # broadcast+clamped diag
        nc.sbuf_tensor([1, LATENT], BF16) as drow,  # raw diag row
        nc.sbuf_tensor([1, P], BF16) as ones,  # PE broadcast stationary
        nc.psum_tensor([P, PSCOL], mybir.dt.float32) as ps,
        nc.semaphore("ls") as ls,  # load completions (+16 each)
        nc.semaphore("ms") as ms,  # mul-drained markers (+1 each)
        nc.semaphore("ss") as ss,  # store completions (+16 each)
        nc.semaphore("bs") as bs,  # diag row DMA (+16)
        nc.semaphore("im") as im,  # ones memset done (+1)
        nc.semaphore("pm") as pm,  # PE matmul round done (+1 each)
        nc.semaphore("dv") as dv,  # DVE psum->sbuf copy done (+1)
    ):
        all_sems = (ls, ms, ss, bs, im, pm, dv)

        # Tiles are split into column chunks for mul/store: full-width
        # [128, chunk] muls use all DVE lanes (a [64, *] row-half runs at
        # half rate), and [128, chunk] stores spread over all 16 SDMA
        # engines. The last tile uses 4 finer chunks to shorten the
        # mul -> store -> receipt tail chain.
        def chunks(i):
            return 4 if i == N_TILES - 1 else 2

        # --- SP engine: x tile loads ---
        for i in range(N_TILES):
            if i >= NBUF:
                # buffer reused: wait for all column-stores of tile i-NBUF
                nc.sync.wait_ge(ss, 32 * (i - NBUF + 1))
            nc.sync.dma_start(out=xbuf[:, buf(i)], in_=xt[i]).then_inc(ls, 16)

        # --- ACT engine: diag row load + column-chunk stores ---
        nc.scalar.dma_start(out=drow[:], in_=d[:]).then_inc(bs, 16)
        gates = 0
        for i in range(N_TILES):
            nch = chunks(i)
            cw = LATENT // nch
            b0 = buf(i).start
            for c in range(nch):
                gates += 1
                nc.scalar.wait_ge(ms, gates)
                nc.scalar.dma_start(
                    out=ot[i][:, c * cw : (c + 1) * cw],
                    in_=xbuf[:, b0 + c * cw : b0 + (c + 1) * cw],
                ).then_inc(ss, 16)

        # --- PE engine: broadcast d across partitions, 2 rounds of 8
        # bank-sized matmuls: ones[1,128].T @ drow[1,512] -> psum[128,512] ---
        nc.tensor.wait_ge(im, 1)
        nc.tensor.wait_ge(bs, 16)
        for r in range(2):
            if r == 1:
                nc.tensor.wait_ge(dv, 1)  # DVE drained round 0 from PSUM
            for c in range(PSCOL // MMCOL):
                col = r * PSCOL + c * MMCOL
                mm = nc.tensor.matmul(
                    out=ps[:, c * MMCOL : (c + 1) * MMCOL],
                    lhsT=ones[:],
                    rhs=drow[:, col : col + MMCOL],
                    start=True,
                    stop=True,
                )
            mm.then_inc(pm, 1)

        # --- DVE engine: ones init, clamp+broadcast copy, then muls ---
        nc.vector.memset(ones[:], 1.0).then_inc(im, 1)
        for r in range(2):
            nc.vector.wait_ge(pm, r + 1)
            # clamp(d, -0.95, 0.95) = min(max(d, -0.95), 0.95) fused into the
            # PSUM -> SBUF bf16 copy
            cc = nc.vector.tensor_scalar(
                out=dbc[:, r * PSCOL : (r + 1) * PSCOL],
                in0=ps[:],
                scalar1=-0.95,
                scalar2=0.95,
                op0=mybir.AluOpType.max,
                op1=mybir.AluOpType.min,
            )
            if r == 0:
                cc.then_inc(dv, 1)
        total_gates = 0
        for i in range(N_TILES):
            nc.vector.wait_ge(ls, 16 * (i + 1))
            nch = chunks(i)
            cw = LATENT // nch
            b0 = buf(i).start
            for c in range(nch):
                cs = slice(b0 + c * cw, b0 + (c + 1) * cw)
                nc.vector.tensor_mul(xbuf[:, cs], xbuf[:, cs], dbc[:, c * cw : (c + 1) * cw])
                # Store-gating inc on a separate tiny DVE op: the per-op DRAIN
                # means it issues only after the mul's writes left the pipe.
                total_gates += 1
                nc.vector.tensor_scalar_mul(dbc[:, 0:1], dbc[:, 0:1], 1.0).then_inc(
                    ms, 1
                )

        # --- tail: reset sems so the NEFF is safely re-executable (NTFF
        # profiling reruns it; leftover sem values would void every wait).
        # No all_engine_barrier (each costs ~3 us of event-butterfly): when
        # ss hits its final value every other engine has already retired its
        # last instruction (ACT's last sem wait is the ms gate of the final
        # store, DVE/SP/PE finish earlier still), so a single gpsimd wait is
        # a sufficient quiesce point, and the runtime re-arms the NEFF only
        # after gpsimd's queue (the last one) completes.
        n_stores = sum(chunks(i) for i in range(N_TILES))
        nc.gpsimd.wait_ge(ss, 16 * n_stores)
        lo = min(s.num for s in all_sems)
        hi = max(s.num for s in all_sems)
        nc.gpsimd.dma_reset(range(lo, hi + 1))
        nc.gpsimd.sem_clear(range(lo, hi + 1))

    _NC_CACHE["nc"] = nc
    return nc


def run(x: np.ndarray, diagonal: np.ndarray, trace: bool = False, **trace_kw):
    """Returns (full_output_f32, BassKernelResults)."""
    x = np.asarray(x, dtype=np.float32)
    diagonal = np.asarray(diagonal, dtype=np.float32)
    assert x.shape == (BATCH, LATENT) and diagonal.shape == (LATENT,)

    nc = _build()
    x16 = x.astype(NP_BF16)
    d16 = diagonal.astype(NP_BF16).reshape(1, LATENT)
    in_maps = [
        {
            "x": x16[c * ROWS_PER_CORE : (c + 1) * ROWS_PER_CORE],
            "diagonal": d16,
        }
        for c in range(N_CORES)
    ]
    res = run_bass_kernel_spmd(
        nc, in_maps, core_ids=list(range(N_CORES)), trace=trace, **trace_kw
    )
    full = np.concatenate(
        [res.results[c]["out"] for c in range(N_CORES)], axis=0
    ).astype(np.float32)
    return full, res


def kernel(x: np.ndarray, diagonal: np.ndarray) -> np.ndarray:
    full, _ = run(x, diagonal, trace=False)
    return full


# revision 7
# speedup vs baseline: 1.0923x; 1.0923x over previous
"""DiagonalLinear on 8 TRN2 NeuronCores.

y = x * clip(diagonal, -0.95, 0.95)  with x [16384, 8192] f32, diagonal [8192] f32.

Purely memory-bound (elementwise): per-core HBM traffic is the whole cost, and
the per-NeuronCore HBM limit is ~358 GB/s shared by loads+stores. The f32
version moves 132 MiB/core (~390 us floor). The rel-err budget (2e-2) is ~10x
wider than a bf16 round-trip (~2e-3), so x is quantized to bf16 on the host,
the device streams bf16 in and out (64 MiB/core), and the host upcasts the
result to f32. That halves HBM bytes -> ~190 us floor.

Data-parallel: x sharded along batch (2048 rows/core). The diagonal is NOT
replicated through HBM (a [128, 8192] bf16 load would cost 2 MiB ~ 6 us of
HBM): instead a single [1, 8192] row (16 KiB) is loaded and broadcast across
the 128 partitions by the otherwise-idle PE (ones[1,128].T @ d[1,512-chunks]
into PSUM), with the clamp fused into the PSUM -> SBUF copy on DVE.

Steady state: 16 tiles of [128, 8192] bf16 (2 MiB contiguous DMAs) stream
through a load -> DVE mul -> store pipeline. Loads issue on the SP HWDGE ring,
stores on the ACT HWDGE ring; both rings feed the same 16 SDMA engines which
round-robin at packet granularity, so the streams share HBM bandwidth without
serializing. Trace shows the DMA union saturated at ~358 GB/s throughout.

Raw Bass (no TileContext): this walrus build rejects Tile's multi-wait
kernel-tail drain, and manual sync keeps every instruction at <=1 sem wait.
The kernel ends with barrier -> sem reset -> barrier so the NEFF is safely
re-executable (NTFF profiling reruns it with leftover sem values otherwise);
the reset is batched into one dma_reset + one sem_clear over the whole sem
range to keep the tail short.
"""

import numpy as np
import ml_dtypes

import concourse.bass as bass
import concourse.mybir as mybir
from concourse.bass_utils import run_bass_kernel_spmd

BATCH = 16384
LATENT = 8192
N_CORES = 8
ROWS_PER_CORE = BATCH // N_CORES  # 2048
P = 128
N_TILES = ROWS_PER_CORE // P  # 16
NBUF = 8
MMCOL = 512  # PE moving-data free-dim max
PSCOL = 4096  # PSUM tensor width (8 banks), half of LATENT

BF16 = mybir.dt.bfloat16
NP_BF16 = ml_dtypes.bfloat16

_NC_CACHE: dict[str, bass.Bass] = {}


def _build() -> bass.Bass:
    if "nc" in _NC_CACHE:
        return _NC_CACHE["nc"]

    nc = bass.Bass()
    x = nc.dram_tensor("x", [ROWS_PER_CORE, LATENT], BF16, kind="ExternalInput")
    d = nc.dram_tensor("diagonal", [1, LATENT], BF16, kind="ExternalInput")
    out = nc.dram_tensor("out", [ROWS_PER_CORE, LATENT], BF16, kind="ExternalOutput")

    xt = x.rearrange("(n p) m -> n p m", p=P)  # [16, 128, 8192]
    ot = out.rearrange("(n p) m -> n p m", p=P)

    def buf(i):
        b = i % NBUF
        return slice(b * LATENT, (b + 1) * LATENT)

    with (
        nc.sbuf_tensor([P, NBUF * LATENT], BF16) as xbuf,
        nc.sbuf_tensor([P, LATENT], BF16) as dbc,  # broadcast+clamped diag
        nc.sbuf_tensor([1, LATENT], BF16) as drow,  # raw diag row
        nc.sbuf_tensor([1, P], BF16) as ones,  # PE broadcast stationary
        nc.psum_tensor([P, PSCOL], mybir.dt.float32) as ps,
        nc.semaphore("ls") as ls,  # load completions (+16 each)
        nc.semaphore("ms") as ms,  # mul-drained markers (+1 each)
        nc.semaphore("ss") as ss,  # store completions (+16 each)
        nc.semaphore("bs") as bs,  # diag row DMA (+16)
        nc.semaphore("im") as im,  # ones memset done (+1)
        nc.semaphore("pm") as pm,  # PE matmul round done (+1 each)
        nc.semaphore("dv") as dv,  # DVE psum->sbuf copy done (+1)
    ):
        all_sems = (ls, ms, ss, bs, im, pm, dv)

        # Tiles are split into column chunks for mul/store: full-width
        # [128, chunk] muls use all DVE lanes (a [64, *] row-half runs at
        # half rate), and [128, chunk] stores spread over all 16 SDMA
        # engines. The last tile uses 4 finer chunks to shorten the
        # mul -> store -> receipt tail chain.
        def chunks(i):
            return 4 if i == N_TILES - 1 else 2

        # --- SP engine: x tile loads ---
        for i in range(N_TILES):
            if i >= NBUF:
                # buffer reused: wait for all column-stores of tile i-NBUF
                nc.sync.wait_ge(ss, 32 * (i - NBUF + 1))
            nc.sync.dma_start(out=xbuf[:, buf(i)], in_=xt[i]).then_inc(ls, 16)

        # --- ACT engine: diag row load + column-chunk stores ---
        nc.scalar.dma_start(out=drow[:], in_=d[:]).then_inc(bs, 16)
        gates = 0
        for i in range(N_TILES):
            nch = chunks(i)
            cw = LATENT // nch
            b0 = buf(i).start
            for c in range(nch):
                gates += 1
                nc.scalar.wait_ge(ms, gates)
                nc.scalar.dma_start(
                    out=ot[i][:, c * cw : (c + 1) * cw],
                    in_=xbuf[:, b0 + c * cw : b0 + (c + 1) * cw],
                ).then_inc(ss, 16)

        # --- PE engine: broadcast d across partitions, 2 rounds of 8
        # bank-sized matmuls: ones[1,128].T @ drow[1,512] -> psum[128,512] ---
        nc.tensor.wait_ge(im, 1)
        nc.tensor.wait_ge(bs, 16)
        for r in range(2):
            if r == 1:
                nc.tensor.wait_ge(dv, 1)  # DVE drained round 0 from PSUM
            for c in range(PSCOL // MMCOL):
                col = r * PSCOL + c * MMCOL
                mm = nc.tensor.matmul(
                    out=ps[:, c * MMCOL : (c + 1) * MMCOL],
                    lhsT=ones[:],
                    rhs=drow[:, col : col + MMCOL],
                    start=True,
                    stop=True,
                )
            mm.then_inc(pm, 1)

        # --- DVE engine: ones init, clamp+broadcast copy, then muls ---
        nc.vector.memset(ones[:], 1.0).then_inc(im, 1)
        for r in range(2):
            nc.vector.wait_ge(pm, r + 1)
            # clamp(d, -0.95, 0.95) = min(max(d, -0.95), 0.95) fused into the
            # PSUM -> SBUF bf16 copy
            cc = nc.vector.tensor_scalar(
                out=dbc[:, r * PSCOL : (r + 1) * PSCOL],
                in0=ps[:],
                scalar1=-0.95,
                scalar2=0.95,
                op0=mybir.AluOpType.max,
                op1=mybir.AluOpType.min,
            )
            if r == 0:
                cc.then_inc(dv, 1)
        total_gates = 0
        for i in range(N_TILES):
            nc.vector.wait_ge(ls, 16 * (i + 1))
            nch = chunks(i)
            cw = LATENT // nch
            b0 = buf(i).start
            for c in range(nch):
                cs = slice(b0 + c * cw, b0 + (c + 1) * cw)
                nc.vector.tensor_mul(xbuf[:, cs], xbuf[:, cs], dbc[:, c * cw : (c + 1) * cw])
                # Store-gating inc on a separate tiny DVE op: the per-op DRAIN
                # means it issues only after the mul's writes left the pipe.
                total_gates += 1
                nc.vector.tensor_scalar_mul(dbc[:, 0:1], dbc[:, 0:1], 1.0).then_inc(
                    ms, 1
                )

        # --- tail: reset sems so the NEFF is safely re-executable (NTFF
        # profiling reruns it; leftover sem values would void every wait).
        # The pre-reset all_engine_barrier (~3 us of event-butterfly) is
        # replaced by a single gpsimd wait: when ss hits its final value
        # every other engine has already retired its last instruction, so
        # gpsimd waiting on ss is a sufficient quiesce point. The POST-reset
        # barrier is required: without it each engine's queue completes
        # before the reset, and on re-execution engines restart with stale
        # semaphore values while the previous iteration's stores are still
        # in flight (measured: ~890k corrupted elements on traced reruns).
        n_stores = sum(chunks(i) for i in range(N_TILES))
        nc.gpsimd.wait_ge(ss, 16 * n_stores)
        lo = min(s.num for s in all_sems)
        hi = max(s.num for s in all_sems)
        nc.gpsimd.dma_reset(range(lo, hi + 1))
        nc.gpsimd.sem_clear(range(lo, hi + 1))
        nc.all_engine_barrier()

    _NC_CACHE["nc"] = nc
    return nc


def run(x: np.ndarray, diagonal: np.ndarray, trace: bool = False, **trace_kw):
    """Returns (full_output_f32, BassKernelResults)."""
    x = np.asarray(x, dtype=np.float32)
    diagonal = np.asarray(diagonal, dtype=np.float32)
    assert x.shape == (BATCH, LATENT) and diagonal.shape == (LATENT,)

    nc = _build()
    x16 = x.astype(NP_BF16)
    d16 = diagonal.astype(NP_BF16).reshape(1, LATENT)
    in_maps = [
        {
            "x": x16[c * ROWS_PER_CORE : (c + 1) * ROWS_PER_CORE],
            "diagonal": d16,
        }
        for c in range(N_CORES)
    ]
    res = run_bass_kernel_spmd(
        nc, in_maps, core_ids=list(range(N_CORES)), trace=trace, **trace_kw
    )
    full = np.concatenate(
        [res.results[c]["out"] for c in range(N_CORES)], axis=0
    ).astype(np.float32)
    return full, res


def kernel(x: np.ndarray, diagonal: np.ndarray) -> np.ndarray:
    full, _ = run(x, diagonal, trace=False)
    return full


# revision 9
# speedup vs baseline: 1.0943x; 1.0018x over previous
"""DiagonalLinear on 8 TRN2 NeuronCores.

y = x * clip(diagonal, -0.95, 0.95)  with x [16384, 8192] f32, diagonal [8192] f32.

Purely memory-bound (elementwise): per-core HBM traffic is the whole cost, and
the per-NeuronCore HBM limit is ~358 GB/s shared by loads+stores. The f32
version moves 132 MiB/core (~390 us floor). The rel-err budget (2e-2) is ~10x
wider than a bf16 round-trip (~2e-3), so x is quantized to bf16 on the host,
the device streams bf16 in and out (64 MiB/core), and the host upcasts the
result to f32. That halves HBM bytes -> ~190 us floor.

Data-parallel: x sharded along batch (2048 rows/core). The diagonal is NOT
replicated through HBM (a [128, 8192] bf16 load would cost 2 MiB ~ 6 us of
HBM): instead a single [1, 8192] row (16 KiB) is loaded and broadcast across
the 128 partitions by the otherwise-idle PE (ones[1,128].T @ d[1,512-chunks]
into PSUM), with the clamp fused into the PSUM -> SBUF copy on DVE.

Steady state: 16 tiles of [128, 8192] bf16 (2 MiB contiguous DMAs) stream
through a load -> DVE mul -> store pipeline. Loads issue on the SP HWDGE ring,
stores on the ACT HWDGE ring; both rings feed the same 16 SDMA engines which
round-robin at packet granularity, so the streams share HBM bandwidth without
serializing. Trace shows the DMA union saturated at ~358 GB/s throughout.

Raw Bass (no TileContext): this walrus build rejects Tile's multi-wait
kernel-tail drain, and manual sync keeps every instruction at <=1 sem wait.
The kernel ends with barrier -> sem reset -> barrier so the NEFF is safely
re-executable (NTFF profiling reruns it with leftover sem values otherwise);
the reset is batched into one dma_reset + one sem_clear over the whole sem
range to keep the tail short.
"""

import numpy as np
import ml_dtypes

import concourse.bass as bass
import concourse.mybir as mybir
from concourse.bass_utils import run_bass_kernel_spmd

BATCH = 16384
LATENT = 8192
N_CORES = 8
ROWS_PER_CORE = BATCH // N_CORES  # 2048
P = 128
N_TILES = ROWS_PER_CORE // P  # 16
NBUF = 8
MMCOL = 512  # PE moving-data free-dim max
PSCOL = 4096  # PSUM tensor width (8 banks), half of LATENT

BF16 = mybir.dt.bfloat16
NP_BF16 = ml_dtypes.bfloat16

_NC_CACHE: dict[str, bass.Bass] = {}


def _build() -> bass.Bass:
    if "nc" in _NC_CACHE:
        return _NC_CACHE["nc"]

    nc = bass.Bass()
    x = nc.dram_tensor("x", [ROWS_PER_CORE, LATENT], BF16, kind="ExternalInput")
    d = nc.dram_tensor("diagonal", [1, LATENT], BF16, kind="ExternalInput")
    out = nc.dram_tensor("out", [ROWS_PER_CORE, LATENT], BF16, kind="ExternalOutput")

    xt = x.rearrange("(n p) m -> n p m", p=P)  # [16, 128, 8192]
    ot = out.rearrange("(n p) m -> n p m", p=P)

    def buf(i):
        b = i % NBUF
        return slice(b * LATENT, (b + 1) * LATENT)

    with (
        nc.sbuf_tensor([P, NBUF * LATENT], BF16) as xbuf,
        nc.sbuf_tensor([P, LATENT], BF16) as dbc,  # broadcast+clamped diag
        nc.sbuf_tensor([1, LATENT], BF16) as drow,  # raw diag row
        nc.sbuf_tensor([1, P], BF16) as ones,  # PE broadcast stationary
        nc.psum_tensor([P, PSCOL], mybir.dt.float32) as ps,
        nc.semaphore("ls") as ls,  # load completions (+16 each)
        nc.semaphore("ms") as ms,  # mul-drained markers (+1 each)
        nc.semaphore("ss") as ss,  # store completions (+16 each)
        nc.semaphore("bs") as bs,  # diag row DMA (+16)
        nc.semaphore("im") as im,  # ones memset done (+1)
        nc.semaphore("pm") as pm,  # PE matmul round done (+1 each)
        nc.semaphore("dv") as dv,  # DVE psum->sbuf copy done (+1)
        nc.semaphore("go") as go,  # iteration gate (gpsimd-owned)
        nc.semaphore("pw") as pw,  # workers-passed-gate counter
    ):
        all_sems = (ls, ms, ss, bs, im, pm, dv, go, pw)

        # Iteration gate: on NEFF re-execution (NTFF profiling reruns the
        # kernel) engine queues can be re-armed independently, so a worker
        # engine could otherwise restart with stale semaphore values while
        # the previous iteration is still draining (measured: ~890k
        # corrupted elements). Every worker's FIRST instruction waits on
        # `go`, which only gpsimd increments — as its own first instruction.
        # Since gpsimd's queue is serial, iteration k+1's `go` cannot rise
        # before iteration k's tail (quiesce + sem reset, below) completed.
        # Mid-iteration, once all 4 workers have checked in via `pw`, gpsimd
        # closes the gate again so early-restarting workers block. This
        # replaces the ~7 us all_engine_barrier tail with ~0.5 us of sem ops.
        nc.gpsimd.sem_inc(go, 1)
        for eng in (nc.sync, nc.scalar, nc.vector, nc.tensor):
            eng.wait_ge(go, 1)
            eng.sem_inc(pw, 1)
        nc.gpsimd.wait_ge(pw, 4)
        nc.gpsimd.sem_clear(go)

        # Tiles are split into column chunks for mul/store: full-width
        # [128, chunk] muls use all DVE lanes (a [64, *] row-half runs at
        # half rate), and [128, chunk] stores spread over all 16 SDMA
        # engines. The last tile uses 4 finer chunks to shorten the
        # mul -> store -> receipt tail chain.
        def chunks(i):
            return 4 if i == N_TILES - 1 else 2

        # --- SP engine: x tile loads ---
        for i in range(N_TILES):
            if i >= NBUF:
                # buffer reused: wait for all column-stores of tile i-NBUF
                nc.sync.wait_ge(ss, 32 * (i - NBUF + 1))
            nc.sync.dma_start(out=xbuf[:, buf(i)], in_=xt[i]).then_inc(ls, 16)

        # --- ACT engine: diag row load + column-chunk stores ---
        nc.scalar.dma_start(out=drow[:], in_=d[:]).then_inc(bs, 16)
        gates = 0
        for i in range(N_TILES):
            nch = chunks(i)
            cw = LATENT // nch
            b0 = buf(i).start
            for c in range(nch):
                gates += 1
                nc.scalar.wait_ge(ms, gates)
                nc.scalar.dma_start(
                    out=ot[i][:, c * cw : (c + 1) * cw],
                    in_=xbuf[:, b0 + c * cw : b0 + (c + 1) * cw],
                ).then_inc(ss, 16)

        # --- PE engine: broadcast d across partitions, 2 rounds of 8
        # bank-sized matmuls: ones[1,128].T @ drow[1,512] -> psum[128,512] ---
        nc.tensor.wait_ge(im, 1)
        nc.tensor.wait_ge(bs, 16)
        for r in range(2):
            if r == 1:
                nc.tensor.wait_ge(dv, 1)  # DVE drained round 0 from PSUM
            for c in range(PSCOL // MMCOL):
                col = r * PSCOL + c * MMCOL
                mm = nc.tensor.matmul(
                    out=ps[:, c * MMCOL : (c + 1) * MMCOL],
                    lhsT=ones[:],
                    rhs=drow[:, col : col + MMCOL],
                    start=True,
                    stop=True,
                )
            mm.then_inc(pm, 1)

        # --- DVE engine: ones init, clamp+broadcast copy, then muls ---
        nc.vector.memset(ones[:], 1.0).then_inc(im, 1)
        for r in range(2):
            nc.vector.wait_ge(pm, r + 1)
            # clamp(d, -0.95, 0.95) = min(max(d, -0.95), 0.95) fused into the
            # PSUM -> SBUF bf16 copy
            cc = nc.vector.tensor_scalar(
                out=dbc[:, r * PSCOL : (r + 1) * PSCOL],
                in0=ps[:],
                scalar1=-0.95,
                scalar2=0.95,
                op0=mybir.AluOpType.max,
                op1=mybir.AluOpType.min,
            )
            if r == 0:
                cc.then_inc(dv, 1)
        total_gates = 0
        for i in range(N_TILES):
            nc.vector.wait_ge(ls, 16 * (i + 1))
            nch = chunks(i)
            cw = LATENT // nch
            b0 = buf(i).start
            for c in range(nch):
                cs = slice(b0 + c * cw, b0 + (c + 1) * cw)
                nc.vector.tensor_mul(xbuf[:, cs], xbuf[:, cs], dbc[:, c * cw : (c + 1) * cw])
                # Store-gating inc on a separate tiny DVE op: the per-op DRAIN
                # means it issues only after the mul's writes left the pipe.
                total_gates += 1
                nc.vector.tensor_scalar_mul(dbc[:, 0:1], dbc[:, 0:1], 1.0).then_inc(
                    ms, 1
                )

        # --- tail: quiesce and reset sems so the NEFF is safely
        # re-executable. When ss hits its final value every other engine has
        # already retired its last instruction and all DMAs have landed, so
        # a single gpsimd wait suffices; the `go` gate above keeps rerun
        # iterations from overlapping this reset. `go` is already 0 here, so
        # clearing the full range is harmless.
        n_stores = sum(chunks(i) for i in range(N_TILES))
        nc.gpsimd.wait_ge(ss, 16 * n_stores)
        lo = min(s.num for s in all_sems)
        hi = max(s.num for s in all_sems)
        nc.gpsimd.dma_reset(range(lo, hi + 1))
        nc.gpsimd.sem_clear(range(lo, hi + 1))

    _NC_CACHE["nc"] = nc
    return nc


def run(x: np.ndarray, diagonal: np.ndarray, trace: bool = False, **trace_kw):
    """Returns (full_output_f32, BassKernelResults)."""
    x = np.asarray(x, dtype=np.float32)
    diagonal = np.asarray(diagonal, dtype=np.float32)
    assert x.shape == (BATCH, LATENT) and diagonal.shape == (LATENT,)

    nc = _build()
    x16 = x.astype(NP_BF16)
    d16 = diagonal.astype(NP_BF16).reshape(1, LATENT)
    in_maps = [
        {
            "x": x16[c * ROWS_PER_CORE : (c + 1) * ROWS_PER_CORE],
            "diagonal": d16,
        }
        for c in range(N_CORES)
    ]
    res = run_bass_kernel_spmd(
        nc, in_maps, core_ids=list(range(N_CORES)), trace=trace, **trace_kw
    )
    full = np.concatenate(
        [res.results[c]["out"] for c in range(N_CORES)], axis=0
    ).astype(np.float32)
    return full, res


def kernel(x: np.ndarray, diagonal: np.ndarray) -> np.ndarray:
    full, _ = run(x, diagonal, trace=False)
    return full
